# revision 1
# baseline (speedup 1.0000x reference)
"""Trainium2 Bass kernel for nn_Attention_81655918231876.

RoPE attention with positional bias, 8 heads / dim_head 64, b=2, n=2048, dim=512.
Sharding: head-parallel across 8 cores. Core h computes head h for BOTH batches
and emits a partial output y_h = softmax(q_h k_h^T + bias_h) v_h @ w_out[h-slice].
The host sums the 8 partials.

Device-side structure (per core):
  - qT/kT/vT [64, n] computed via matmuls (weights stationary, x^T moving),
    RoPE folded in via host-prebuilt rotated weight columns + cos/sin tables.
  - exp(S + bias) is computed as exp(S) * exp(bias): the host sends
    ebT = exp(pos_bias[h]).T and the multiply rides the PSUM->SBUF copy of
    the transposed P blocks (one DVE tensor_tensor instead of a copy), so the
    positional bias costs no extra PE or DVE passes.
  - V is reconstituted in natural [n, d] layout from vT by PE transposes; an
    extra ones-column in V makes the O^T = V_ext^T P^T accumulation emit the
    softmax row sums as row 64 of O^T for free. 1/sum is folded into the
    final y projection as a per-partition tensor_scalar multiply.
  - All matmuls use float32r (single-pass fp32 PE mode); transposes too.
"""

import numpy as np
import ml_dtypes
import sys

sys.path.insert(0, "/opt/trn_rl_repo")

HEADS = 8
DIM_HEAD = 64
ROPE_THETA = 10000.0
B, N, DIM = 2, 2048, 512
NB = N // 128  # 16 i-blocks

_compiled = None


def _build():
    import concourse.bass as bass
    import concourse.tile as tile
    from concourse import bacc, mybir

    f32 = mybir.dt.float32
    f32r = mybir.dt.float32r
    Exp = mybir.ActivationFunctionType.Exp

    nc = bacc.Bacc(None, target_bir_lowering=False, debug=False)
    xt = nc.dram_tensor("xt", [DIM, 2 * N], f32r, kind="ExternalInput")
    wall = nc.dram_tensor("wall", [DIM, 384], f32r, kind="ExternalInput")
    cs2 = nc.dram_tensor("cs2", [128, N], f32, kind="ExternalInput")
    bf16 = mybir.dt.bfloat16
    ebt = nc.dram_tensor("ebt", [N, N], f32, kind="ExternalInput")
    wo = nc.dram_tensor("wo", [64, DIM], f32r, kind="ExternalInput")
    idr = nc.dram_tensor("idr", [128, 128], f32r, kind="ExternalInput")
    out = nc.dram_tensor("out", [B, N, DIM], f32, kind="ExternalOutput")

    with tile.TileContext(nc) as tc:
        with (
            tc.tile_pool(name="singles", bufs=1) as singles,
            tc.tile_pool(name="xtp", bufs=4) as xtp,
            tc.tile_pool(name="ebp", bufs=17) as ebp,
            tc.tile_pool(name="pp", bufs=2) as pp,
            tc.tile_pool(name="ptp", bufs=16) as ptp,
            tc.tile_pool(name="t1p", bufs=2) as t1p,
            tc.tile_pool(name="yp", bufs=3) as yp,
            tc.tile_pool(name="psA", bufs=2, space="PSUM") as psA,
            tc.tile_pool(name="psB", bufs=2, space="PSUM") as psB,
            tc.tile_pool(name="psC", bufs=2, space="PSUM") as psC,
        ):
            # constants
            wl_sb = [singles.tile([128, 384], f32r, tag=f"wl{k}", name=f"wl_sb{k}") for k in range(4)]
            for k in range(4):
                nc.sync.dma_start(out=wl_sb[k], in_=wall[128 * k:128 * (k + 1), :])
            cs_sb = singles.tile([128, N], f32, tag="cs")
            nc.sync.dma_start(out=cs_sb, in_=cs2[:, :])
            idr_sb = singles.tile([128, 128], f32r, tag="idr")
            nc.sync.dma_start(out=idr_sb, in_=idr[:, :])
            wo_sb = singles.tile([64, DIM], f32r, tag="wo")
            nc.sync.dma_start(out=wo_sb, in_=wo[:, :])
            vinit = nc.dram_tensor("vinit", [128, 16 * 128], f32r, kind="ExternalInput")

            for b in range(B):
                # ---- projection phase ----
                xb = [xtp.tile([128, N], f32r, tag="xt", name=f"xb{b}_{_k}") for _k in range(4)]
                for k in range(4):
                    nc.sync.dma_start(
                        out=xb[k], in_=xt[128 * k:128 * (k + 1), b * N:(b + 1) * N]
                    )
                qhh = singles.tile([128, N], bf16, tag="qhh")
                qlo = singles.tile([64, N], bf16, tag="qlo")
                kpack = singles.tile([128, N], bf16, tag="kpack")
                vt = singles.tile([128, N], f32r, tag="vt")
                for mt in (0, 1):
                    for nch in range(4):
                        ps = psA.tile([128, 512], f32, tag="s")
                        for k in range(4):
                            nc.tensor.matmul(
                                ps,
                                wl_sb[k][:, 128 * mt:128 * (mt + 1)],
                                xb[k][:, 512 * nch:512 * (nch + 1)],
                                start=(k == 0),
                                stop=(k == 3),
                            )
                        sl = slice(512 * nch, 512 * (nch + 1))
                        t1 = t1p.tile([64, 512], f32, tag="t1")
                        t2 = t1p.tile([64, 512], f32, tag="t2")
                        nc.vector.tensor_mul(t1, ps[0:64, :], cs_sb[0:64, sl])
                        nc.vector.tensor_mul(t2, ps[64:128, :], cs_sb[64:128, sl])
                        qk = t1p.tile([64, 512], f32, tag="qk")
                        nc.vector.tensor_add(qk, t1, t2)
                        mul, add = mybir.AluOpType.mult, mybir.AluOpType.add
                        if mt == 0:
                            nc.vector.tensor_copy(qhh[0:64, sl], qk)
                            nc.vector.tensor_copy(qhh[64:128, sl], qk)
                            nc.vector.scalar_tensor_tensor(
                                qlo[:, sl], qhh[0:64, sl], -1.0, qk, mul, add)
                        else:
                            nc.vector.tensor_copy(kpack[0:64, sl], qk)
                            nc.vector.scalar_tensor_tensor(
                                kpack[64:128, sl], kpack[0:64, sl], -1.0, qk, mul, add)
                for nch in range(4):
                    ps = psA.tile([128, 512], f32, tag="s")
                    for k in range(4):
                        nc.tensor.matmul(
                            ps,
                            wl_sb[k][:, 256:384],
                            xb[k][:, 512 * nch:512 * (nch + 1)],
                            start=(k == 0),
                            stop=(k == 3),
                        )
                    nc.vector.tensor_copy(vt[:, 512 * nch:512 * (nch + 1)], ps)
                # V natural layout [n, d] tiles (+ ones column at col 64)
                vsb = singles.tile([128, 16 * 128], f32r, tag="vv")
                nc.sync.dma_start(out=vsb, in_=vinit[:, :])
                for jt in range(16):
                    vq = psB.tile([128, 128], f32r, tag="pt")
                    nc.tensor.transpose(
                        vq, vt[:, 128 * jt:128 * (jt + 1)], idr_sb
                    )
                    nc.vector.tensor_copy(
                        vsb[:, 128 * jt:128 * jt + 64], vq[:, 0:64]
                    )

                # ---- attention phase ----
                rinv_u = singles.tile([128, NB], f32, tag="rinv")
                otsb = singles.tile([128, N], f32r, tag="ot")
                for isup in range(4):
                    ptile = [ptp.tile([128, 512], f32r, tag="pt", name=f"ptile_{b}_{isup}_{_j}") for _j in range(16)]
                    ebtile = [ebp.tile([128, 512], f32, tag="eb", name=f"ebt_{b}_{isup}_{_j}") for _j in range(16)]
                    for jt in range(16):
                        nc.sync.dma_start(
                            out=ebtile[jt],
                            in_=ebt[128 * jt:128 * (jt + 1), 512 * isup:512 * (isup + 1)],
                        )
                    i0 = isup * 512
                    for jbp in range(8):
                        s_ps = psA.tile([128, 1024], f32, tag="s")
                        for hh in range(2):
                            jb = 2 * jbp + hh
                            osl = slice(512 * hh, 512 * (hh + 1))
                            jsl = slice(128 * jb, 128 * (jb + 1))
                            nc.tensor.matmul(
                                s_ps[:, osl], kpack[:, jsl], qhh[:, i0:i0 + 512],
                                start=True, stop=False,
                            )
                            nc.tensor.matmul(
                                s_ps[:, osl], kpack[0:64, jsl], qlo[:, i0:i0 + 512],
                                start=False, stop=True,
                            )
                        pts = pp.tile([128, 1024], f32r, tag="p")
                        nc.scalar.activation(pts, s_ps, Exp)
                        for hh in range(2):
                            jb = 2 * jbp + hh
                            nc.vector.tensor_mul(
                                ptile[jb],
                                pts[:, 512 * hh:512 * (hh + 1)],
                                ebtile[jb],
                            )
                    ot_ps = psC.tile([128, 512], f32, tag="o")
                    for jt in range(16):
                        nc.tensor.matmul(
                            ot_ps,
                            vsb[:, 128 * jt:128 * (jt + 1)],
                            ptile[jt],
                            start=(jt == 0),
                            stop=(jt == 15),
                        )
                    nc.vector.tensor_copy(otsb[:, 512 * isup:512 * (isup + 1)], ot_ps)

                # ---- normalization + output projection ----
                for tb in range(NB):
                    tps = psB.tile([128, 128], f32r, tag="pt")
                    nc.tensor.transpose(
                        tps, otsb[:, 128 * tb:128 * (tb + 1)], idr_sb
                    )
                    nc.vector.reciprocal(
                        rinv_u[:, tb:tb + 1], tps[:, 64:65].bitcast(f32)
                    )
                for ib in range(NB):
                    y_ps = psC.tile([128, 512], f32, tag="o")
                    nc.tensor.matmul(
                        y_ps, otsb[0:64, 128 * ib:128 * (ib + 1)], wo_sb,
                        start=True, stop=True,
                    )
                    y_sb = yp.tile([128, 512], f32, tag="y")
                    nc.vector.tensor_scalar_mul(y_sb, y_ps, rinv_u[:, ib:ib + 1])
                    nc.sync.dma_start(
                        out=out[b, 128 * ib:128 * (ib + 1), :], in_=y_sb
                    )

    nc.compile()
    return nc


def _host_inputs(x, pos_bias, w_qkv, w_out):
    """Build the per-core input maps (head-parallel sharding)."""
    x = np.asarray(x, dtype=np.float32)
    pos_bias = np.asarray(pos_bias, dtype=np.float32)
    w_qkv = np.asarray(w_qkv, dtype=np.float32)
    w_out = np.asarray(w_out, dtype=np.float32)
    hidden = HEADS * DIM_HEAD

    xt = np.ascontiguousarray(np.concatenate([x[0].T, x[1].T], axis=1))  # [512, 4096]

    inv_freq = 1.0 / (ROPE_THETA ** (np.arange(0, DIM_HEAD, 2, dtype=np.float64) / DIM_HEAD))
    freqs = np.arange(N, dtype=np.float64)[:, None] * inv_freq[None, :]
    freqs = np.repeat(freqs, 2, axis=-1)  # [n, 64]
    cosT = np.cos(freqs).T.astype(np.float32)
    sinT = np.sin(freqs).T.astype(np.float32)
    cs2 = np.ascontiguousarray(np.concatenate([cosT, sinT], axis=0))  # [128, n]

    def rot_cols(w):
        wr = np.empty_like(w)
        wr[:, 0::2] = -w[:, 1::2]
        wr[:, 1::2] = w[:, 0::2]
        return wr

    def _vinit():
        v = np.zeros((128, 16 * 128), dtype=np.float32)
        v[:, 64::128] = 1.0
        return v

    scale = DIM_HEAD ** -0.5
    ident = np.eye(128, dtype=np.float32)
    in_maps = []
    for h in range(HEADS):
        wq = w_qkv[:, h * 64:(h + 1) * 64] * scale
        wk = w_qkv[:, hidden + h * 64:hidden + (h + 1) * 64]
        wvh = w_qkv[:, 2 * hidden + h * 64:2 * hidden + (h + 1) * 64]
        wall = np.ascontiguousarray(
            np.concatenate(
                [wq, rot_cols(wq), wk, rot_cols(wk), wvh,
                 np.zeros((DIM, 64), dtype=np.float32)], axis=1)
        )  # [512, 384]
        in_maps.append({
            "xt": xt,
            "wall": wall,
            "cs2": cs2,
            "ebt": np.ascontiguousarray(np.exp(pos_bias[h]).T),
            "wo": np.ascontiguousarray(w_out[h * 64:(h + 1) * 64, :]),
            "idr": ident,
            "vinit": _vinit(),
        })
    return in_maps


def kernel(x, pos_bias, w_qkv, w_out, _want_trace=False):
    global _compiled
    from concourse.bass_utils import run_bass_kernel_spmd

    if _compiled is None:
        _compiled = _build()
    in_maps = _host_inputs(x, pos_bias, w_qkv, w_out)
    res = run_bass_kernel_spmd(
        _compiled, in_maps, core_ids=list(range(HEADS)), trace=_want_trace
    )
    y = np.zeros((B, N, DIM), dtype=np.float32)
    for r in res.results:
        y += r["out"]
    if _want_trace:
        kernel._last_results = res
    return y



# revision 15
# speedup vs baseline: 1.3177x; 1.3177x over previous
"""Trainium2 Bass kernel for nn_Attention_81655918231876.

RoPE attention with positional bias, 8 heads / dim_head 64, b=2, n=2048, dim=512.
Sharding: head-parallel across 8 cores. Core h computes head h for BOTH batches
and emits a partial output y_h = softmax(q_h k_h^T + bias_h) v_h @ w_out[h-slice].
The host sums the 8 partials.

v2 design (all-bf16 matmul path; ~50x accuracy headroom under the 2e-2 gate):
  - Projections: stationary weight blocks [q|qrot], [k|krot], [v|pad] in bf16,
    moving x^T chunks; RoPE combine = one DVE mul with a stacked cos/sin table
    plus one DVE add that writes bf16 q/k packs ([b0;b1] on partitions).
  - S = q k^T as plain bf16 K=64 matmuls (tile_position rows 0/64 pick the
    batch half of the packed q/k tiles).
  - exp(S) on ScalarE -> bf16; bias multiply exp(S)*exp(bias) on DVE at 2x
    bf16 rate against a resident bf16 exp(bias^T) table loaded once.
  - P V accumulated with an extra ones-column in V so row 64 of O^T is the
    softmax row sum; 1/sum broadcast across partitions (gpsimd) and folded
    into O^T before the output projection; y DMA'd straight from PSUM.
  - V natural layout built with DMA xbar transposes (no PE transposes).
  - S -> exp -> mult -> PV software-pipelined with a 2-step lag so the PE
    stream never waits on ScalarE/DVE (keeps the PE p-state at full clock).
"""

import numpy as np
import ml_dtypes
import sys

sys.path.insert(0, "/opt/trn_rl_repo")

HEADS = 8
DIM_HEAD = 64
ROPE_THETA = 10000.0
B, N, DIM = 2, 2048, 512
# per-j-block column stride in vsb: 64 V cols + 1 ones col + pad. Must keep
# every block's byte offset 32B-aligned: the DMA xbar transpose writes in
# 16-element (bf16) groups and silently corrupts unaligned destinations.
VSTRIDE = 80

_compiled = None
_DEBUG = False


def _build():
    import concourse.bass as bass
    import concourse.tile as tile
    from concourse import bacc, mybir

    f32 = mybir.dt.float32
    bf16 = mybir.dt.bfloat16
    Exp = mybir.ActivationFunctionType.Exp
    Copy = mybir.ActivationFunctionType.Copy

    nc = bacc.Bacc(None, target_bir_lowering=False, debug=False)
    xt = nc.dram_tensor("xt", [DIM, 2 * N], bf16, kind="ExternalInput")
    wall = nc.dram_tensor("wall", [DIM, 384], bf16, kind="ExternalInput")
    cs2 = nc.dram_tensor("cs2", [128, N], f32, kind="ExternalInput")
    ebt = nc.dram_tensor("ebt", [N, N], bf16, kind="ExternalInput")
    wo = nc.dram_tensor("wo", [64, DIM], bf16, kind="ExternalInput")
    out = nc.dram_tensor("out", [B, N, DIM], bf16, kind="ExternalOutput")
    if _DEBUG:
        dbg_qkv = nc.dram_tensor("dbg_qkv", [3, 128, N], bf16, kind="ExternalOutput")
        dbg_vsb = nc.dram_tensor("dbg_vsb", [B, 128, 16 * VSTRIDE], bf16, kind="ExternalOutput")
        dbg_pt = nc.dram_tensor("dbg_pt", [128, 1024], bf16, kind="ExternalOutput")
        dbg_rb = nc.dram_tensor("dbg_rb", [64, 512], f32, kind="ExternalOutput")

    with tile.TileContext(nc) as tc:
        with (
            tc.tile_pool(name="singles", bufs=1) as singles,
            tc.tile_pool(name="t12p", bufs=3) as t12p,
            tc.tile_pool(name="ptsp", bufs=3) as ptsp,
            tc.tile_pool(name="ptp", bufs=6) as ptp,
            tc.tile_pool(name="rrp", bufs=2) as rrp,
            tc.tile_pool(name="otp", bufs=2) as otp,
            tc.tile_pool(name="ysp", bufs=3) as ysp,
        ):
            # ---- constants / inputs ----
            wl = [singles.tile([128, 384], bf16, tag=f"wl{k}", name=f"wl{k}") for k in range(4)]
            for k in range(4):
                nc.sync.dma_start(out=wl[k], in_=wall[128 * k:128 * (k + 1), :])
            xb = [singles.tile([128, 2 * N], bf16, tag=f"xb{k}", name=f"xb{k}") for k in range(4)]
            for half in range(2):
                for k in range(4):
                    nc.sync.dma_start(
                        out=xb[k][:, N * half:N * (half + 1)],
                        in_=xt[128 * k:128 * (k + 1), N * half:N * (half + 1)],
                    )
            cs_sb = singles.tile([128, N], f32, tag="cs", name="cs_sb")
            nc.sync.dma_start(out=cs_sb, in_=cs2[:, :])
            wo_sb = singles.tile([64, DIM], bf16, tag="wo", name="wo_sb")
            nc.sync.dma_start(out=wo_sb, in_=wo[:, :])
            eb_sb = singles.tile([128, 16 * N], bf16, tag="eb", name="eb_sb")
            for j in range(16):
                nc.sync.dma_start(
                    out=eb_sb[:, N * j:N * (j + 1)],
                    in_=ebt[128 * j:128 * (j + 1), :],
                )

            qb = singles.tile([128, N], bf16, tag="qb", name="qb")
            kb = singles.tile([128, N], bf16, tag="kb", name="kb")
            vt = singles.tile([128, N], bf16, tag="vt", name="vt")
            vsb = [singles.tile([128, 16 * VSTRIDE], bf16, tag=f"vsb{b}", name=f"vsb{b}")
                   for b in range(B)]
            for b in range(B):
                nc.vector.memset(vsb[b], 1.0)

            # ---- projection phase ----
            with tc.tile_pool(name="psP", bufs=6, space="PSUM") as psP:
                for mt in range(3):  # 0: q|qrot, 1: k|krot, 2: v|pad
                    for half in range(2):
                        chunks = [4 * half + c for c in range(4)]
                        tiles = [psP.tile([128, 512], f32, tag="s",
                                          name=f"pp_{mt}_{half}_{ci}")
                                 for ci in range(4)]
                        for k in range(4):
                            for ci, c in enumerate(chunks):
                                nc.tensor.matmul(
                                    tiles[ci],
                                    wl[k][:, 128 * mt:128 * (mt + 1)],
                                    xb[k][:, 512 * c:512 * (c + 1)],
                                    start=(k == 0), stop=(k == 3),
                                )
                        for ci, c in enumerate(chunks):
                            b = c // 4
                            tok = 512 * (c % 4)
                            if mt < 2:
                                t1 = t12p.tile([64, 512], f32, tag="t1",
                                               name=f"t1_{mt}_{c}")
                                t2 = t12p.tile([64, 512], f32, tag="t2",
                                               name=f"t2_{mt}_{c}")
                                nc.vector.tensor_mul(t1, tiles[ci][0:64, :],
                                                     cs_sb[0:64, tok:tok + 512])
                                nc.vector.tensor_mul(t2, tiles[ci][64:128, :],
                                                     cs_sb[64:128, tok:tok + 512])
                                dst = qb if mt == 0 else kb
                                nc.gpsimd.tensor_add(
                                    dst[64 * b:64 * b + 64, tok:tok + 512],
                                    t1, t2)
                            else:
                                nc.scalar.activation(
                                    vt[64 * b:64 * b + 64, tok:tok + 512],
                                    tiles[ci][0:64, :], Copy)

            # V natural layout via DMA xbar transposes: vt[64b:64b+64, jblk]
            # -> vsb[b][:, VSTRIDE*j : +64]  (ones column at +64 from memset)
            for b in range(B):
                for j in range(16):
                    nc.sync.dma_start_transpose(
                        vsb[b][:, VSTRIDE * j:VSTRIDE * j + 64],
                        vt[64 * b:64 * b + 64, 128 * j:128 * (j + 1)],
                    )

            if _DEBUG:
                nc.sync.dma_start(out=dbg_qkv[0, :, :], in_=qb)
                nc.sync.dma_start(out=dbg_qkv[1, :, :], in_=kb)
                nc.sync.dma_start(out=dbg_qkv[2, :, :], in_=vt)
                for b in range(B):
                    nc.sync.dma_start(out=dbg_vsb[b, :, :], in_=vsb[b])

            # ---- attention ----
            # Both batches processed together per (i-quarter, j): the two
            # K=64 S matmuls land on PE row-groups 0/64 and run concurrently.
            with (
                tc.tile_pool(name="psS", bufs=2, space="PSUM") as psS,
                tc.tile_pool(name="psO", bufs=1, space="PSUM") as psO,
                tc.tile_pool(name="psY", bufs=2, space="PSUM") as psY,
            ):
                def attn_quarter(q, fillers):
                    """Emit one 512-token i-quarter (both batches); returns
                    deferred normalization + output-projection closures."""
                    i0 = 512 * q
                    fill_iter = iter(fillers)

                    def emit_fill():
                        f = next(fill_iter, None)
                        if f is not None:
                            f()

                    ots = [psO.tile([65, 512], f32, tag=f"o{b}", name=f"ot_{b}_{q}")
                           for b in range(B)]
                    pt_tiles = {}
                    for step in range(18):
                        if step < 16:
                            j = step
                            s_ps = psS.tile([128, 1024], f32, tag="s",
                                            name=f"s_{q}_{j}")
                            for b in range(B):
                                nc.tensor.matmul(
                                    s_ps[:, 512 * b:512 * (b + 1)],
                                    kb[64 * b:64 * b + 64, 128 * j:128 * (j + 1)],
                                    qb[64 * b:64 * b + 64, i0:i0 + 512],
                                    start=True, stop=True,
                                )
                            pts = ptsp.tile([128, 1024], bf16, tag="pts",
                                            name=f"pts_{q}_{j}")
                            nc.scalar.activation(pts, s_ps, Exp)
                            pt = ptp.tile([128, 1024], bf16, tag="pt",
                                          name=f"pt_{q}_{j}")
                            ebs = eb_sb[:, N * j + i0:N * j + i0 + 512]
                            for b in range(B):
                                nc.vector.tensor_mul(
                                    pt[:, 512 * b:512 * (b + 1)],
                                    pts[:, 512 * b:512 * (b + 1)], ebs)
                            pt_tiles[j] = pt
                            if _DEBUG and q == 0 and j == 0:
                                nc.sync.dma_start(out=dbg_pt[:, :], in_=pt)
                        emit_fill()
                        if step >= 2:
                            j = step - 2
                            for b in range(B):
                                nc.tensor.matmul(
                                    ots[b],
                                    vsb[b][:, VSTRIDE * j:VSTRIDE * j + 65],
                                    pt_tiles[j][:, 512 * b:512 * (b + 1)],
                                    start=(j == 0), stop=(j == 15),
                                )
                            pt_tiles[j] = None
                    for f in fill_iter:
                        f()

                    # 1/rowsum from the ones-column row; broadcast across
                    # partitions on gpsimd; fold the scale into the O^T
                    # PSUM->SBUF copy (per-token scale varies along free dim)
                    deferred = []
                    for b in range(B):
                        ot = ots[b]
                        rr = rrp.tile([1, 512], f32, tag="rr", name=f"rr_{b}_{q}")
                        nc.vector.reciprocal(rr, ot[64:65, :])
                        rb = rrp.tile([64, 512], f32, tag="rb", name=f"rb_{b}_{q}")
                        nc.gpsimd.partition_broadcast(rb, rr)
                        if _DEBUG and q == 0 and b == 0:
                            nc.sync.dma_start(out=dbg_rb[:, :], in_=rb)
                        otsb = otp.tile([64, 512], bf16, tag=f"otsb{b}",
                                        name=f"otsb_{b}_{q}")

                        def mk_oscale(ot=ot, otsb=otsb, rb=rb):
                            def f():
                                nc.vector.tensor_mul(otsb, ot[0:64, :], rb)
                            return f

                        def mk_y(blk, otsb=otsb, b=b):
                            def f():
                                y_ps = psY.tile([128, 512], f32, tag="y",
                                                name=f"y_{b}_{q}_{blk}")
                                nc.tensor.matmul(
                                    y_ps, otsb[:, 128 * blk:128 * (blk + 1)],
                                    wo_sb, start=True, stop=True)
                                y_sb = ysp.tile([128, 512], bf16, tag="ysb",
                                                name=f"ysb_{b}_{q}_{blk}")
                                if blk % 2 == 1:
                                    nc.scalar.activation(y_sb, y_ps, Copy)
                                else:
                                    nc.vector.tensor_copy(y_sb, y_ps)
                                nc.sync.dma_start(
                                    out=out[b, i0 + 128 * blk:i0 + 128 * (blk + 1), :],
                                    in_=y_sb)
                            return f

                        deferred.append(mk_oscale())
                        deferred += [mk_y(blk) for blk in range(4)]
                    return deferred

                deferred = []
                for q in range(4):
                    deferred = attn_quarter(q, deferred)
                for f in deferred:
                    f()

    nc.compile()
    return nc


def _host_inputs(x, pos_bias, w_qkv, w_out):
    """Build the per-core input maps (head-parallel sharding)."""
    bf = ml_dtypes.bfloat16
    x = np.asarray(x, dtype=np.float32)
    pos_bias = np.asarray(pos_bias, dtype=np.float32)
    w_qkv = np.asarray(w_qkv, dtype=np.float32)
    w_out = np.asarray(w_out, dtype=np.float32)
    hidden = HEADS * DIM_HEAD

    xt = np.ascontiguousarray(
        np.concatenate([x[0].T, x[1].T], axis=1)).astype(bf)  # [512, 4096]

    inv_freq = 1.0 / (ROPE_THETA ** (np.arange(0, DIM_HEAD, 2, dtype=np.float64) / DIM_HEAD))
    freqs = np.arange(N, dtype=np.float64)[:, None] * inv_freq[None, :]
    freqs = np.repeat(freqs, 2, axis=-1)  # [n, 64]
    cosT = np.cos(freqs).T.astype(np.float32)
    sinT = np.sin(freqs).T.astype(np.float32)
    cs2 = np.ascontiguousarray(np.concatenate([cosT, sinT], axis=0))  # [128, n]

    def rot_cols(w):
        wr = np.empty_like(w)
        wr[:, 0::2] = -w[:, 1::2]
        wr[:, 1::2] = w[:, 0::2]
        return wr

    scale = DIM_HEAD ** -0.5
    in_maps = []
    for h in range(HEADS):
        wq = w_qkv[:, h * 64:(h + 1) * 64] * scale
        wk = w_qkv[:, hidden + h * 64:hidden + (h + 1) * 64]
        wvh = w_qkv[:, 2 * hidden + h * 64:2 * hidden + (h + 1) * 64]
        wall = np.ascontiguousarray(
            np.concatenate(
                [wq, rot_cols(wq), wk, rot_cols(wk), wvh,
                 np.zeros((DIM, 64), dtype=np.float32)], axis=1)
        ).astype(bf)  # [512, 384]
        in_maps.append({
            "xt": xt,
            "wall": wall,
            "cs2": cs2,
            "ebt": np.ascontiguousarray(np.exp(pos_bias[h]).T).astype(bf),
            "wo": np.ascontiguousarray(w_out[h * 64:(h + 1) * 64, :]).astype(bf),
        })
    return in_maps


def kernel(x, pos_bias, w_qkv, w_out, _want_trace=False):
    global _compiled
    from concourse.bass_utils import run_bass_kernel_spmd

    if _compiled is None:
        _compiled = _build()
    in_maps = _host_inputs(x, pos_bias, w_qkv, w_out)
    res = run_bass_kernel_spmd(
        _compiled, in_maps, core_ids=list(range(HEADS)), trace=_want_trace
    )
    y = np.zeros((B, N, DIM), dtype=np.float32)
    for r in res.results:
        y += r["out"]
    if _want_trace:
        kernel._last_results = res
    return y


# revision 25
# speedup vs baseline: 1.5724x; 1.1932x over previous
"""Trainium2 Bass kernel for nn_Attention_81655918231876.

RoPE attention with positional bias, 8 heads / dim_head 64, b=2, n=2048, dim=512.
Sharding: head-parallel across 8 cores. Core h computes head h for BOTH batches
and emits a partial output y_h = softmax(q_h k_h^T + bias_h) v_h @ w_out[h-slice].
The host sums the 8 partials.

v2 design (all-bf16 matmul path; ~50x accuracy headroom under the 2e-2 gate):
  - Projections: stationary weight blocks [q|qrot], [k|krot], [v|pad] in bf16,
    moving x^T chunks; RoPE combine = one DVE mul with a stacked cos/sin table
    plus one DVE add that writes bf16 q/k packs ([b0;b1] on partitions).
  - S = q k^T as plain bf16 K=64 matmuls (tile_position rows 0/64 pick the
    batch half of the packed q/k tiles).
  - exp(S) on ScalarE -> bf16; bias multiply exp(S)*exp(bias) on DVE at 2x
    bf16 rate against a resident bf16 exp(bias^T) table loaded once.
  - P V accumulated with an extra ones-column in V so row 64 of O^T is the
    softmax row sum; 1/sum broadcast across partitions (gpsimd) and folded
    into O^T before the output projection; y DMA'd straight from PSUM.
  - V natural layout built with DMA xbar transposes (no PE transposes).
  - S -> exp -> mult -> PV software-pipelined with a 2-step lag so the PE
    stream never waits on ScalarE/DVE (keeps the PE p-state at full clock).
"""

import numpy as np
import ml_dtypes
import sys

sys.path.insert(0, "/opt/trn_rl_repo")

HEADS = 8
DIM_HEAD = 64
ROPE_THETA = 10000.0
B, N, DIM = 2, 2048, 512
# per-j-block column stride in vsb: 64 V cols + 1 ones col + pad. Must keep
# every block's byte offset 32B-aligned: the DMA xbar transpose writes in
# 16-element (bf16) groups and silently corrupts unaligned destinations.
VSTRIDE = 80

_compiled = None
_DEBUG = False


def _build():
    import concourse.bass as bass
    import concourse.tile as tile
    from concourse import bacc, mybir

    f32 = mybir.dt.float32
    bf16 = mybir.dt.bfloat16
    Exp = mybir.ActivationFunctionType.Exp
    Copy = mybir.ActivationFunctionType.Copy

    nc = bacc.Bacc(None, target_bir_lowering=False, debug=False)
    xt = nc.dram_tensor("xt", [DIM, 2 * N], bf16, kind="ExternalInput")
    wall = nc.dram_tensor("wall", [DIM, 384], bf16, kind="ExternalInput")
    cs2 = nc.dram_tensor("cs2", [128, N], f32, kind="ExternalInput")
    ebt = nc.dram_tensor("ebt", [N, N], bf16, kind="ExternalInput")
    wo = nc.dram_tensor("wo", [64, DIM], bf16, kind="ExternalInput")
    out = nc.dram_tensor("out", [B, N, DIM], bf16, kind="ExternalOutput")
    rsum = nc.dram_tensor("rsum", [4 * B, 512], f32, kind="ExternalOutput")
    if _DEBUG:
        dbg_qkv = nc.dram_tensor("dbg_qkv", [3, 128, N], bf16, kind="ExternalOutput")
        dbg_vsb = nc.dram_tensor("dbg_vsb", [B, 128, 16 * VSTRIDE], bf16, kind="ExternalOutput")
        dbg_pt = nc.dram_tensor("dbg_pt", [128, 1024], bf16, kind="ExternalOutput")
        dbg_rb = nc.dram_tensor("dbg_rb", [64, 512], f32, kind="ExternalOutput")

    with tile.TileContext(nc) as tc:
        with (
            tc.tile_pool(name="singles", bufs=1) as singles,
            tc.tile_pool(name="t12p", bufs=3) as t12p,
            tc.tile_pool(name="ptsp", bufs=3) as ptsp,
            tc.tile_pool(name="ptp", bufs=6) as ptp,
            tc.tile_pool(name="rrp", bufs=2) as rrp,
            tc.tile_pool(name="otp", bufs=2) as otp,
            tc.tile_pool(name="ysp", bufs=3) as ysp,
        ):
            # ---- constants / inputs ----
            wl = [singles.tile([128, 384], bf16, tag=f"wl{k}", name=f"wl{k}") for k in range(4)]
            for k in range(4):
                nc.sync.dma_start(out=wl[k], in_=wall[128 * k:128 * (k + 1), :])
            xb = [singles.tile([128, 2 * N], bf16, tag=f"xb{k}", name=f"xb{k}") for k in range(4)]
            for half in range(2):
                for k in range(4):
                    nc.sync.dma_start(
                        out=xb[k][:, N * half:N * (half + 1)],
                        in_=xt[128 * k:128 * (k + 1), N * half:N * (half + 1)],
                    )
            cs_sb = singles.tile([128, N], f32, tag="cs", name="cs_sb")
            nc.sync.dma_start(out=cs_sb, in_=cs2[:, :])
            wo_sb = singles.tile([64, DIM], bf16, tag="wo", name="wo_sb")
            nc.sync.dma_start(out=wo_sb, in_=wo[:, :])
            eb_sb = singles.tile([128, 16 * N], bf16, tag="eb", name="eb_sb")
            for j in range(16):
                nc.sync.dma_start(
                    out=eb_sb[:, N * j:N * (j + 1)],
                    in_=ebt[128 * j:128 * (j + 1), :],
                )

            qb = singles.tile([128, N], bf16, tag="qb", name="qb")
            kb = singles.tile([128, N], bf16, tag="kb", name="kb")
            vt = singles.tile([128, N], bf16, tag="vt", name="vt")
            vsb = [singles.tile([128, 16 * VSTRIDE], bf16, tag=f"vsb{b}", name=f"vsb{b}")
                   for b in range(B)]
            for b in range(B):
                nc.vector.memset(vsb[b], 1.0)

            # ---- projection phase ----
            with tc.tile_pool(name="psP", bufs=6, space="PSUM") as psP:
                for mt in range(3):  # 0: q|qrot, 1: k|krot, 2: v|pad
                    for half in range(2):
                        chunks = [4 * half + c for c in range(4)]
                        tiles = [psP.tile([128, 512], f32, tag="s",
                                          name=f"pp_{mt}_{half}_{ci}")
                                 for ci in range(4)]
                        for k in range(4):
                            for ci, c in enumerate(chunks):
                                nc.tensor.matmul(
                                    tiles[ci],
                                    wl[k][:, 128 * mt:128 * (mt + 1)],
                                    xb[k][:, 512 * c:512 * (c + 1)],
                                    start=(k == 0), stop=(k == 3),
                                )
                        for ci, c in enumerate(chunks):
                            b = c // 4
                            tok = 512 * (c % 4)
                            if mt < 2:
                                t1 = t12p.tile([64, 512], f32, tag="t1",
                                               name=f"t1_{mt}_{c}")
                                t2 = t12p.tile([64, 512], f32, tag="t2",
                                               name=f"t2_{mt}_{c}")
                                nc.vector.tensor_mul(t1, tiles[ci][0:64, :],
                                                     cs_sb[0:64, tok:tok + 512])
                                nc.vector.tensor_mul(t2, tiles[ci][64:128, :],
                                                     cs_sb[64:128, tok:tok + 512])
                                dst = qb if mt == 0 else kb
                                nc.gpsimd.tensor_add(
                                    dst[64 * b:64 * b + 64, tok:tok + 512],
                                    t1, t2)
                            else:
                                nc.scalar.activation(
                                    vt[64 * b:64 * b + 64, tok:tok + 512],
                                    tiles[ci][0:64, :], Copy)

            # V natural layout via DMA xbar transposes (per 128-token block;
            # 32B-aligned destinations): vt[64b:64b+64, jblk] -> vsb[b] block
            for b in range(B):
                for j in range(16):
                    nc.sync.dma_start_transpose(
                        vsb[b][:, VSTRIDE * j:VSTRIDE * j + 64],
                        vt[64 * b:64 * b + 64, 128 * j:128 * (j + 1)],
                    )

            if _DEBUG:
                nc.sync.dma_start(out=dbg_qkv[0, :, :], in_=qb)
                nc.sync.dma_start(out=dbg_qkv[1, :, :], in_=kb)
                nc.sync.dma_start(out=dbg_qkv[2, :, :], in_=vt)
                for b in range(B):
                    nc.sync.dma_start(out=dbg_vsb[b, :, :], in_=vsb[b])

            # ---- attention ----
            # Both batches processed together per (i-quarter, j): the two
            # K=64 S matmuls land on PE row-groups 0/64 and run concurrently.
            with (
                tc.tile_pool(name="psS", bufs=2, space="PSUM") as psS,
                tc.tile_pool(name="psO", bufs=1, space="PSUM") as psO,
                tc.tile_pool(name="psY", bufs=2, space="PSUM") as psY,
            ):
                def attn_quarter(q, fillers):
                    """Emit one 512-token i-quarter (both batches); returns
                    deferred normalization + output-projection closures."""
                    i0 = 512 * q
                    fill_iter = iter(fillers)

                    def emit_fill():
                        f = next(fill_iter, None)
                        if f is not None:
                            f()

                    ots = [psO.tile([65, 512], f32, tag=f"o{b}", name=f"ot_{b}_{q}")
                           for b in range(B)]
                    pt_tiles = {}
                    for step in range(18):
                        if step < 16:
                            j = step
                            s_ps = psS.tile([128, 1024], f32, tag="s",
                                            name=f"s_{q}_{j}")
                            for b in range(B):
                                nc.tensor.matmul(
                                    s_ps[:, 512 * b:512 * (b + 1)],
                                    kb[64 * b:64 * b + 64, 128 * j:128 * (j + 1)],
                                    qb[64 * b:64 * b + 64, i0:i0 + 512],
                                    start=True, stop=True,
                                )
                            pts = ptsp.tile([128, 1024], bf16, tag="pts",
                                            name=f"pts_{q}_{j}")
                            nc.scalar.activation(pts, s_ps, Exp)
                            pt = ptp.tile([128, 1024], bf16, tag="pt",
                                          name=f"pt_{q}_{j}")
                            ebs = eb_sb[:, N * j + i0:N * j + i0 + 512]
                            for b in range(B):
                                nc.vector.tensor_mul(
                                    pt[:, 512 * b:512 * (b + 1)],
                                    pts[:, 512 * b:512 * (b + 1)], ebs)
                            pt_tiles[j] = pt
                            if _DEBUG and q == 0 and j == 0:
                                nc.sync.dma_start(out=dbg_pt[:, :], in_=pt)
                        emit_fill()
                        if step >= 2:
                            j = step - 2
                            for b in range(B):
                                nc.tensor.matmul(
                                    ots[b],
                                    vsb[b][:, VSTRIDE * j:VSTRIDE * j + 65],
                                    pt_tiles[j][:, 512 * b:512 * (b + 1)],
                                    start=(j == 0), stop=(j == 15),
                                )
                            pt_tiles[j] = None
                    for f in fill_iter:
                        f()

                    # rowsums ship to the host (f32); y goes out unnormalized
                    # in bf16 and the host divides by the per-head rowsum.
                    deferred = []
                    for b in range(B):
                        ot = ots[b]
                        rs = rrp.tile([1, 512], f32, tag="rs", name=f"rs_{b}_{q}")
                        nc.vector.tensor_copy(rs, ot[64:65, :])
                        nc.sync.dma_start(out=rsum[4 * b + q:4 * b + q + 1, :],
                                          in_=rs)
                        otsb = otp.tile([64, 512], bf16, tag=f"otsb{b}",
                                        name=f"otsb_{b}_{q}")
                        nc.vector.tensor_copy(otsb, ot[0:64, :])

                        def mk_y(blk, otsb=otsb, b=b):
                            def f():
                                y_ps = psY.tile([128, 512], f32, tag="y",
                                                name=f"y_{b}_{q}_{blk}")
                                nc.tensor.matmul(
                                    y_ps, otsb[:, 128 * blk:128 * (blk + 1)],
                                    wo_sb, start=True, stop=True)
                                y_sb = ysp.tile([128, 512], bf16, tag="ysb",
                                                name=f"ysb_{b}_{q}_{blk}")
                                if blk % 2 == 1:
                                    nc.scalar.activation(y_sb, y_ps, Copy)
                                else:
                                    nc.vector.tensor_copy(y_sb, y_ps)
                                nc.sync.dma_start(
                                    out=out[b, i0 + 128 * blk:i0 + 128 * (blk + 1), :],
                                    in_=y_sb)
                            return f

                        deferred += [mk_y(blk) for blk in range(4)]
                    return deferred

                deferred = []
                for q in range(4):
                    deferred = attn_quarter(q, deferred)
                for f in deferred:
                    f()

    nc.compile()
    return nc


def _host_inputs(x, pos_bias, w_qkv, w_out):
    """Build the per-core input maps (head-parallel sharding)."""
    bf = ml_dtypes.bfloat16
    x = np.asarray(x, dtype=np.float32)
    pos_bias = np.asarray(pos_bias, dtype=np.float32)
    w_qkv = np.asarray(w_qkv, dtype=np.float32)
    w_out = np.asarray(w_out, dtype=np.float32)
    hidden = HEADS * DIM_HEAD

    xt = np.ascontiguousarray(
        np.concatenate([x[0].T, x[1].T], axis=1)).astype(bf)  # [512, 4096]

    inv_freq = 1.0 / (ROPE_THETA ** (np.arange(0, DIM_HEAD, 2, dtype=np.float64) / DIM_HEAD))
    freqs = np.arange(N, dtype=np.float64)[:, None] * inv_freq[None, :]
    freqs = np.repeat(freqs, 2, axis=-1)  # [n, 64]
    cosT = np.cos(freqs).T.astype(np.float32)
    sinT = np.sin(freqs).T.astype(np.float32)
    cs2 = np.ascontiguousarray(np.concatenate([cosT, sinT], axis=0))  # [128, n]

    def rot_cols(w):
        wr = np.empty_like(w)
        wr[:, 0::2] = -w[:, 1::2]
        wr[:, 1::2] = w[:, 0::2]
        return wr

    scale = DIM_HEAD ** -0.5
    in_maps = []
    for h in range(HEADS):
        wq = w_qkv[:, h * 64:(h + 1) * 64] * scale
        wk = w_qkv[:, hidden + h * 64:hidden + (h + 1) * 64]
        wvh = w_qkv[:, 2 * hidden + h * 64:2 * hidden + (h + 1) * 64]
        wall = np.ascontiguousarray(
            np.concatenate(
                [wq, rot_cols(wq), wk, rot_cols(wk), wvh,
                 np.zeros((DIM, 64), dtype=np.float32)], axis=1)
        ).astype(bf)  # [512, 384]
        in_maps.append({
            "xt": xt,
            "wall": wall,
            "cs2": cs2,
            "ebt": np.ascontiguousarray(np.exp(pos_bias[h]).T).astype(bf),
            "wo": np.ascontiguousarray(w_out[h * 64:(h + 1) * 64, :]).astype(bf),
        })
    return in_maps


def kernel(x, pos_bias, w_qkv, w_out, _want_trace=False):
    global _compiled
    from concourse.bass_utils import run_bass_kernel_spmd

    if _compiled is None:
        _compiled = _build()
    in_maps = _host_inputs(x, pos_bias, w_qkv, w_out)
    res = run_bass_kernel_spmd(
        _compiled, in_maps, core_ids=list(range(HEADS)), trace=_want_trace
    )
    y = np.zeros((B, N, DIM), dtype=np.float32)
    for r in res.results:
        rs = np.asarray(r["rsum"]).reshape(B, N)
        y += r["out"].astype(np.float32) / rs[:, :, None]
    if _want_trace:
        kernel._last_results = res
    return y
